# revision 22
# baseline (speedup 1.0000x reference)
"""Trainium2 Bass kernel for nn_Memory (scatter_memory).

Contract: kernel(query, keys_b) -> (updated_query, updated_memory_b,
score_memory_b, loss_list), matching reference.py semantics up to fp32
rounding.

Sharding: data-parallel over batch B across 8 cores (2 batches / core).
The only cross-core quantity is the memory update (a segment-sum over all
B*H*W queries weighted by exp(score - colmax)).  We factor
    wsel[n] = exp(s1[n] - gmax[top1[n]]) = exp(s1[n]) * exp(-gmax[top1[n]])
so each core emits   S_c[m, :] = sum_n onehot[n,m] * exp(s1[n]) * q[n, :]
plus its local column-max; the host reduces gmax = max_c colmax_c and forms
query_update = exp(-gmax)[:, None] * sum_c S_c.  No device collectives.

Top-1/top-2 handling is index-free: one-hot masks are built by exact fp32
equality against the row max / second max (scores are recomputed transposed
with identical accumulation order, so equality is bitwise-safe), and the
per-row key statistics (|k|^2, sum k) needed by the losses are gathered with
tiny mask @ table matmuls.
"""

import sys

sys.path.insert(0, "/opt/trn_rl_repo")

import numpy as np

import concourse.bass as bass
import concourse.mybir as mybir
from concourse import bacc
from concourse.masks import make_identity
from concourse.tile import TileContext

# Problem constants (hardcoded per harness contract)
B, D, H, W = 16, 512, 32, 32
M = 1024
NCORES = 8
BL = B // NCORES          # batches per core = 2
HW = H * W                # 1024
NL = BL * HW              # rows per core = 2048
NT = NL // 128            # 16 n-tiles
MC = M // 128             # 8 m-chunks
KC = D // 128             # 4 k-chunks (contraction over d)
DC = D // 128             # 4 d-chunks
NQ = NL // 512            # 4 n-quarters (stage C)
EPS_NORM = 1e-12
EPS_TRIP = 1e-6
MARGIN = 1.0
F32 = mybir.dt.float32
NEG_INF = -3.0e38
import os
STAGES = set(os.environ.get("BASS_STAGES", "123456ABCED"))
AOT = mybir.AluOpType
AFT = mybir.ActivationFunctionType


def build_nc():
    nc = bacc.Bacc(None, target_bir_lowering=False)

    qin = nc.dram_tensor("qin", [BL, D, HW], F32, kind="ExternalInput")
    keys = nc.dram_tensor("keys", [M, D], F32, kind="ExternalInput")

    uq = nc.dram_tensor("uq", [BL, 2 * D, HW], F32, kind="ExternalOutput")
    smb = nc.dram_tensor("smb", [NL, M], F32, kind="ExternalOutput")
    s_out = nc.dram_tensor("s_out", [M, D], F32, kind="ExternalOutput")
    cmax_o = nc.dram_tensor("cmax_o", [128, MC], F32, kind="ExternalOutput")
    loss_o = nc.dram_tensor("loss_o", [BL, 2], F32, kind="ExternalOutput")

    # DRAM bounce for partition->free flattens ([k,128] -> [1, 128k])
    flat_dr = nc.dram_tensor("flat_dr", [64, 128], F32)

    with TileContext(nc) as tc:
        _body(tc, qin, keys, uq, smb, s_out, cmax_o, loss_o, flat_dr)
    return nc


def _body(tc, qin, keys, uq, smb, s_out, cmax_o, loss_o, flat_dr):
    nc = tc.nc
    from contextlib import ExitStack

    with ExitStack() as ctx:
        persist = ctx.enter_context(tc.tile_pool(name="persist", bufs=1))
        stage = ctx.enter_context(tc.tile_pool(name="stage", bufs=3))
        rows = ctx.enter_context(tc.tile_pool(name="rows", bufs=3))

        # ---- persistent SBUF tensors ----
        qT = [persist.tile([128, NL], F32, tag=f"qT{kc}", name=f"qT{kc}")
              for kc in range(KC)]
        mask1 = [persist.tile([128, M], F32, tag=f"mask1_{nt}", name=f"mask1_{nt}")
                 for nt in range(NT)]
        top8 = persist.tile([128, 8 * NT], F32, tag="top8", name="top8")
        cm_acc = persist.tile([128, M], F32, tag="cm_acc", name="cm_acc")
        tabs = persist.tile([128, 2 * MC], F32, tag="tabs", name="tabs")
        ident = persist.tile([128, 128], F32, tag="ident", name="ident")
        ones_col = persist.tile([128, 1], F32, tag="ones_col", name="ones_col")
        ones_row = persist.tile([1, 128], F32, tag="ones_row", name="ones_row")

        # [128, NT] per-row stats (n = nt*128 + p)
        def st(tag):
            return persist.tile([128, NT], F32, tag=tag, name=tag)

        s1_nm, s2_nm, negs1_nm, rsum_nm, rinv_nm = (
            st("s1"), st("s2"), st("negs1"), st("rsum"), st("rinv"))
        ssq_nm, inv_nm, qsum_nm, w_nm = st("ssq"), st("inv"), st("qsum"), st("w")
        kn1_nm, ks1_nm, kn2_nm, ks2_nm = st("kn1"), st("ks1"), st("kn2"), st("ks2")

        # [1, NL] rows (free-layout), rotating 3 slots:
        # ssq->0 qs->1 inv->2 (stage A only), then negb->0 c1->1 c2->2 (B..C)
        def rw(name):
            return rows.tile([1, NL], F32, tag="rowbuf", name=name)

        ssq_row, qs_row, inv_row = rw("ssq_row"), rw("qs_row"), rw("inv_row")
        c1_row, c2_row = rw("c1_row"), rw("c2_row")

        make_identity(nc, ident[:])
        nc.vector.memset(ones_col[:], 1.0)
        nc.vector.memset(ones_row[:], 1.0)
        nc.vector.memset(cm_acc[:], NEG_INF)

        # m-index column table: mcol_f[p, mc] = mc*128 + p (exact in f32)
        mcol_i = persist.tile([128, MC], mybir.dt.int32, tag="mcol_i",
                              name="mcol_i")
        mcol_f = persist.tile([128, MC], F32, tag="mcol_f", name="mcol_f")
        nc.gpsimd.iota(mcol_i[:], pattern=[[128, MC]], base=0,
                       channel_multiplier=1)
        nc.vector.tensor_copy(out=mcol_f[:], in_=mcol_i[:])
        idx8 = persist.tile([128, 8 * NT], mybir.dt.uint32, tag="idx8",
                            name="idx8")
        idx1f, idx2f = st("idx1f"), st("idx2f")

        with tc.tile_pool(name="pool_k", bufs=1) as pool_k:
            keys_sb = [pool_k.tile([128, D], F32, tag=f"keys{mc}", name=f"keys{mc}")
                       for mc in range(MC)]
            keysT = [pool_k.tile([128, M], F32, tag=f"keysT{kc}", name=f"keysT{kc}")
                     for kc in range(KC)]

            # ============ Stage A: loads, keysT, tables, normalize ============
            with tc.tile_pool(name="psA", bufs=1, space="PSUM") as psA:
              if "1" in STAGES:
                for mc in range(MC):
                    nc.sync.dma_start(out=keys_sb[mc][:],
                                      in_=keys[mc * 128:(mc + 1) * 128, :])
                    # tables: kn = sum_d k^2, ks = sum_d k
                    if "5" in STAGES:
                        sq_junk = stage.tile([128, D], F32, tag="stA", name="sq_junk")
                        nc.scalar.square(out=sq_junk[:], in_=keys_sb[mc][:])
                        nc.vector.reduce_sum(
                            out=tabs[:, 2 * mc:2 * mc + 1], in_=sq_junk[:],
                            axis=mybir.AxisListType.X)
                        nc.vector.reduce_sum(
                            out=tabs[:, 2 * mc + 1:2 * mc + 2], in_=keys_sb[mc][:],
                            axis=mybir.AxisListType.X)
                    if "6" not in STAGES:
                        continue
                    for dc in range(DC):
                        ktp = psA.tile([128, 128], F32, tag="ktp", name="ktp",
                                       bufs=2)
                        nc.tensor.transpose(
                            out=ktp[:], in_=keys_sb[mc][:, dc * 128:(dc + 1) * 128],
                            identity=ident[:])
                        nc.scalar.copy(out=keysT[dc][:, mc * 128:(mc + 1) * 128],
                                       in_=ktp[:])

              if "2" in STAGES:
                # raw qT load (d-major: partition=d, free = b*HW + hw)
                for kc in range(KC):
                    for b in range(BL):
                        nc.sync.dma_start(
                            out=qT[kc][:, b * HW:(b + 1) * HW],
                            in_=qin[b, kc * 128:(kc + 1) * 128, :])

              if "3" in STAGES:
                # ssq + raw column sums via ones-matmuls
                for nq in range(NQ):
                    ps_ssq = psA.tile([1, 512], F32, tag="ps_ssq", name="ps_ssq")
                    ps_qs = psA.tile([1, 512], F32, tag="ps_qs", name="ps_qs")
                    for kc in range(KC):
                        sq_t = stage.tile([128, 512], F32, tag="stA", name="sq_t")
                        nc.scalar.square(out=sq_t[:],
                                         in_=qT[kc][:, nq * 512:(nq + 1) * 512])
                        nc.tensor.matmul(
                            out=ps_ssq[:], lhsT=ones_col[:], rhs=sq_t[:],
                            start=(kc == 0), stop=(kc == KC - 1))
                        nc.tensor.matmul(
                            out=ps_qs[:], lhsT=ones_col[:],
                            rhs=qT[kc][:, nq * 512:(nq + 1) * 512],
                            start=(kc == 0), stop=(kc == KC - 1))
                    nc.scalar.copy(out=ssq_row[:, nq * 512:(nq + 1) * 512],
                                   in_=ps_ssq[:])
                    nc.scalar.copy(out=qs_row[:, nq * 512:(nq + 1) * 512],
                                   in_=ps_qs[:])

                # unflatten ssq/qs rows -> [128, NT] n-major (DRAM bounce + PE T)
                nc.sync.dma_start(
                    out=flat_dr[0:16, :].rearrange("t p -> () (t p)"),
                    in_=ssq_row[:])
                nc.sync.dma_start(
                    out=flat_dr[16:32, :].rearrange("t p -> () (t p)"),
                    in_=qs_row[:])
                unf = stage.tile([32, 128], F32, tag="unf", name="unf", bufs=1)
                nc.sync.dma_start(out=unf[:], in_=flat_dr[0:32, :])
                unfp = psA.tile([128, 32], F32, tag="unfp", name="unfp")
                nc.tensor.transpose(out=unfp[:], in_=unf[:],
                                    identity=ident[0:32, 0:32])
                nc.scalar.copy(out=ssq_nm[:], in_=unfp[:, 0:16])
                qsraw_nm = persist.tile([128, NT], F32, tag="qsraw", name="qsraw")
                nc.scalar.copy(out=qsraw_nm[:], in_=unfp[:, 16:32])

                # inv = 1/max(sqrt(ssq), eps); qsum = qsraw * inv
                nc.scalar.sqrt(out=inv_nm[:], in_=ssq_nm[:])
                nc.vector.tensor_scalar_max(inv_nm[:], inv_nm[:], EPS_NORM)
                nc.vector.reciprocal(out=inv_nm[:], in_=inv_nm[:])
                nc.vector.tensor_tensor(out=qsum_nm[:], in0=qsraw_nm[:],
                                        in1=inv_nm[:], op=AOT.mult)

                # flatten inv_nm -> inv_row
                invp = psA.tile([16, 128], F32, tag="invp", name="invp")
                nc.tensor.transpose(out=invp[:], in_=inv_nm[:], identity=ident[:])
                invT = stage.tile([16, 128], F32, tag="invT", name="invT", bufs=1)
                nc.scalar.copy(out=invT[:], in_=invp[:])
                nc.sync.dma_start(out=flat_dr[32:48, :], in_=invT[:])
                nc.sync.dma_start(
                    out=inv_row[:],
                    in_=flat_dr[32:48, :].rearrange("t p -> () (t p)"))

              if "4" in STAGES:
                # normalize qT in place: qT *= bcast(inv_row)
                for nq in range(NQ):
                    binv_t = stage.tile([128, 512], F32, tag="stA", name="binv_t")
                    nc.gpsimd.partition_broadcast(binv_t[:],
                                                  inv_row[:, nq * 512:(nq + 1) * 512])
                    for kc in range(KC):
                        nc.vector.tensor_tensor(
                            out=qT[kc][:, nq * 512:(nq + 1) * 512],
                            in0=qT[kc][:, nq * 512:(nq + 1) * 512],
                            in1=binv_t[:], op=AOT.mult)

                # updated_query first half: channels [0, D) = normalized q
                for dc in range(DC):
                    for b in range(BL):
                        nc.sync.dma_start(
                            out=uq[b, dc * 128:(dc + 1) * 128, :],
                            in_=qT[dc][:, b * HW:(b + 1) * HW])

            # ============ Stage B: score [n, m], row stats ============
            with tc.tile_pool(name="psB", bufs=3, space="PSUM") as psB:
              if 'B' in STAGES:
                for nt in range(NT):
                    ps_s = psB.tile([128, M], F32, tag="score", name="score")
                    for kc in range(KC):
                        for mh in range(2):
                            nc.tensor.matmul(
                                out=ps_s[:, mh * 512:(mh + 1) * 512],
                                lhsT=qT[kc][:, nt * 128:(nt + 1) * 128],
                                rhs=keysT[kc][:, mh * 512:(mh + 1) * 512],
                                start=(kc == 0), stop=(kc == KC - 1))
                    # top-8 values (descending): s1 = col 0, s2 = col 1
                    nc.vector.max(out=top8[:, nt * 8:(nt + 1) * 8], in_=ps_s[:])
                    nc.vector.max_index(
                        out=idx8[:, nt * 8:(nt + 1) * 8],
                        in_max=top8[:, nt * 8:(nt + 1) * 8], in_values=ps_s[:])
                    nc.vector.tensor_scalar_mul(
                        negs1_nm[:, nt:nt + 1], top8[:, nt * 8:nt * 8 + 1], -1.0)
                    # one-hot top1 mask (scatter weights)
                    nc.vector.tensor_scalar(
                        out=mask1[nt][:], in0=ps_s[:],
                        scalar1=top8[:, nt * 8:nt * 8 + 1], scalar2=None,
                        op0=AOT.is_equal)
                    # column-max accumulation (raw scores)
                    nc.vector.tensor_tensor(
                        out=cm_acc[:], in0=cm_acc[:], in1=ps_s[:], op=AOT.max)
                    # exp(score - s1) with fused row-sum
                    sm_t = stage.tile([128, M], F32, tag="sm_t", name="sm_t",
                                      bufs=2)
                    nc.scalar.activation(
                        out=sm_t[:], in_=ps_s[:], func=AFT.Exp,
                        bias=negs1_nm[:, nt:nt + 1], scale=1.0,
                        accum_out=rsum_nm[:, nt:nt + 1])
                    nc.vector.reciprocal(
                        out=rinv_nm[:, nt:nt + 1], in_=rsum_nm[:, nt:nt + 1])
                    nc.scalar.mul(out=sm_t[:], in_=sm_t[:],
                                  mul=rinv_nm[:, nt:nt + 1])
                    nc.sync.dma_start(out=smb[nt * 128:(nt + 1) * 128, :],
                                      in_=sm_t[:])

                # post-loop row stats
                t8v = top8[:].rearrange("p (t e) -> p t e", e=8)
                nc.vector.tensor_copy(
                    out=s1_nm[:].rearrange("p (t e) -> p t e", e=1),
                    in_=t8v[:, :, 0:1])
                nc.vector.tensor_copy(
                    out=s2_nm[:].rearrange("p (t e) -> p t e", e=1),
                    in_=t8v[:, :, 1:2])
                # w = exp(s1) (scatter row weights)
                nc.scalar.activation(out=w_nm[:], in_=s1_nm[:], func=AFT.Exp)
                i8v = idx8[:].rearrange("p (t e) -> p t e", e=8)
                nc.vector.tensor_copy(
                    out=idx1f[:].rearrange("p (t e) -> p t e", e=1),
                    in_=i8v[:, :, 0:1])
                nc.vector.tensor_copy(
                    out=idx2f[:].rearrange("p (t e) -> p t e", e=1),
                    in_=i8v[:, :, 1:2])

            # flatten negb/c1/c2 -> rows (one PE transpose + DRAM bounce)
            with tc.tile_pool(name="psF", bufs=1, space="PSUM") as psF:
              if "B" in STAGES:
                stk = stage.tile([128, 32], F32, tag="stk", name="stk", bufs=1)
                nc.vector.tensor_copy(out=stk[:, 0:16], in_=idx1f[:])
                nc.vector.tensor_copy(out=stk[:, 16:32], in_=idx2f[:])
                stkp = psF.tile([32, 128], F32, tag="stkp", name="stkp")
                nc.tensor.transpose(out=stkp[:], in_=stk[:],
                                    identity=ident[0:128, 0:128])
                stkT = stage.tile([32, 128], F32, tag="stkT", name="stkT", bufs=1)
                nc.scalar.copy(out=stkT[:], in_=stkp[:])
                nc.sync.dma_start(out=flat_dr[0:32, :], in_=stkT[:])
                nc.sync.dma_start(
                    out=c1_row[:],
                    in_=flat_dr[0:16, :].rearrange("t p -> () (t p)"))
                nc.sync.dma_start(
                    out=c2_row[:],
                    in_=flat_dr[16:32, :].rearrange("t p -> () (t p)"))

            # ==== Stage C: score^T [m, n] w/ bias; read-matmul; lookups ====
            lk_acc = persist.tile([128, 4 * NT], F32, tag="lk_acc", name="lk_acc")
            nc.vector.memset(lk_acc[:], 0.0)
            with tc.tile_pool(name="psC", bufs=1, space="PSUM") as psC:
              if "C" in STAGES:
                for nq in range(NQ):
                    nsl = slice(nq * 512, (nq + 1) * 512)
                    # broadcast index rows for this n-quarter (gpsimd)
                    bc1_t = stage.tile([128, 512], F32, tag="stBc", name="bc1_t",
                                       bufs=4)
                    bc2_t = stage.tile([128, 512], F32, tag="stBc", name="bc2_t",
                                       bufs=4)
                    nc.gpsimd.partition_broadcast(bc1_t[:], c1_row[:, nsl])
                    nc.gpsimd.partition_broadcast(bc2_t[:], c2_row[:, nsl])

                    pcs = [psC.tile([128, 512], F32, tag=f"pc{dc}",
                                    name=f"pc{dc}") for dc in range(DC)]
                    for mc in range(MC):
                        # masks: m == top_k index (exact integer compare)
                        m1t = stage.tile([128, 512], F32, tag="stC", name="m1t",
                                         bufs=6)
                        m2t = stage.tile([128, 512], F32, tag="stC", name="m2t",
                                         bufs=6)
                        nc.vector.tensor_scalar(
                            out=m1t[:], in0=bc1_t[:],
                            scalar1=mcol_f[:, mc:mc + 1], scalar2=None,
                            op0=AOT.is_equal)
                        nc.vector.tensor_scalar(
                            out=m2t[:], in0=bc2_t[:],
                            scalar1=mcol_f[:, mc:mc + 1], scalar2=None,
                            op0=AOT.is_equal)
                        # score_memory^T chunk: re-read smb rows, PE-transpose
                        smT_t = stage.tile([128, 512], F32, tag="stC",
                                           name="smT_t", bufs=6)
                        for ns in range(4):
                            t = nq * 4 + ns
                            smn = stage.tile([128, 128], F32, tag="smn",
                                             name="smn", bufs=4)
                            nc.sync.dma_start(
                                out=smn[:],
                                in_=smb[t * 128:(t + 1) * 128,
                                        mc * 128:(mc + 1) * 128])
                            ptr = psC.tile([128, 128], F32, tag="ptr",
                                           name="ptr", bufs=3)
                            nc.tensor.transpose(out=ptr[:], in_=smn[:],
                                                identity=ident[:])
                            eng = nc.scalar if ns % 2 == 0 else nc.vector
                            if ns % 2 == 0:
                                nc.scalar.copy(
                                    out=smT_t[:, ns * 128:(ns + 1) * 128],
                                    in_=ptr[:])
                            else:
                                nc.vector.tensor_copy(
                                    out=smT_t[:, ns * 128:(ns + 1) * 128],
                                    in_=ptr[:])
                        # table lookups: out [n 128, 2] per ns; groups close
                        # immediately, accumulation over mc happens in SBUF
                        lk_t = psC.tile([128, 16], F32, tag="lk_t", name="lk_t",
                                        bufs=1)
                        for ns in range(4):
                            nc.tensor.matmul(
                                out=lk_t[:, 4 * ns:4 * ns + 2],
                                lhsT=m1t[:, ns * 128:(ns + 1) * 128],
                                rhs=tabs[:, 2 * mc:2 * mc + 2],
                                start=True, stop=True)
                            nc.tensor.matmul(
                                out=lk_t[:, 4 * ns + 2:4 * ns + 4],
                                lhsT=m2t[:, ns * 128:(ns + 1) * 128],
                                rhs=tabs[:, 2 * mc:2 * mc + 2],
                                start=True, stop=True)
                        nc.vector.tensor_tensor(
                            out=lk_acc[:, 16 * nq:16 * (nq + 1)],
                            in0=lk_acc[:, 16 * nq:16 * (nq + 1)],
                            in1=lk_t[:], op=AOT.add)
                        # read: out[d, n] += keys[m, d]^T @ smT[m, n]
                        for dc in range(DC):
                            nc.tensor.matmul(
                                out=pcs[dc][:],
                                lhsT=keys_sb[mc][:, dc * 128:(dc + 1) * 128],
                                rhs=smT_t[:], start=(mc == 0),
                                stop=(mc == MC - 1))
                    b = nq // 2
                    hsl = slice((nq % 2) * 512, (nq % 2) * 512 + 512)
                    for dc in range(DC):
                        cc_t = stage.tile([128, 512], F32, tag="stO",
                                          name="cc_t")
                        nc.scalar.copy(out=cc_t[:], in_=pcs[dc][:])
                        nc.sync.dma_start(
                            out=uq[b, D + dc * 128:D + (dc + 1) * 128, hsl],
                            in_=cc_t[:])

                # materialize lookup stats [128, NT]
                lkv = lk_acc[:].rearrange("p (t e) -> p t e", e=4)
                for dst, i in ((kn1_nm, 0), (ks1_nm, 1), (kn2_nm, 2),
                               (ks2_nm, 3)):
                    nc.vector.tensor_copy(
                        out=dst[:].rearrange("p (t e) -> p t e", e=1),
                        in_=lkv[:, :, i:i + 1])

        # ============ Stage E: scatter-sum S[m, d] ============
        with tc.tile_pool(name="pool_qn", bufs=1) as pool_qn, \
             tc.tile_pool(name="psE", bufs=2, space="PSUM") as psE:
          if "E" in STAGES:
            # qn[nt] = transpose(qT normalized) * exp(s1) per row
            qn = [pool_qn.tile([128, D], F32, tag=f"qn{nt}", name=f"qn{nt}")
                  for nt in range(NT)]
            for nt in range(NT):
                for dc in range(DC):
                    qtp = psE.tile([128, 128], F32, tag="qtp", name="qtp")
                    nc.tensor.transpose(
                        out=qtp[:], in_=qT[dc][:, nt * 128:(nt + 1) * 128],
                        identity=ident[:])
                    nc.scalar.mul(out=qn[nt][:, dc * 128:(dc + 1) * 128],
                                  in_=qtp[:], mul=w_nm[:, nt:nt + 1])
            for mc in range(MC):
                ps_S = psE.tile([128, D], F32, tag="ps_S", name="ps_S")
                for nt in range(NT):
                    nc.tensor.matmul(
                        out=ps_S[:],
                        lhsT=mask1[nt][:, mc * 128:(mc + 1) * 128],
                        rhs=qn[nt][:], start=(nt == 0), stop=(nt == NT - 1))
                s_t = stage.tile([128, D], F32, tag="stO", name="s_t")
                nc.scalar.copy(out=s_t[:], in_=ps_S[:])
                nc.sync.dma_start(out=s_out[mc * 128:(mc + 1) * 128, :],
                                  in_=s_t[:])

        # ============ Stage D: losses + colmax out ============
        with tc.tile_pool(name="psD", bufs=2, space="PSUM") as psD:
          if "D" in STAGES:
            cmax_sb = persist.tile([128, MC], F32, tag="cmax_sb", name="cmax_sb")
            for mc in range(MC):
                cmp_ = psD.tile([128, 128], F32, tag="cmp", name="cmp")
                nc.tensor.transpose(
                    out=cmp_[:], in_=cm_acc[:, mc * 128:(mc + 1) * 128],
                    identity=ident[:])
                nc.vector.reduce_max(out=cmax_sb[:, mc:mc + 1], in_=cmp_[:],
                                     axis=mybir.AxisListType.X)
            nc.sync.dma_start(out=cmax_o[:], in_=cmax_sb[:])

            def pst(tag):
                return persist.tile([128, NT], F32, tag=tag, name=tag)

            mse, tmp, dap, dan, trip = (pst("mse"), pst("tmp"), pst("dap"),
                                        pst("dan"), pst("trip"))
            # mse = 1 - 2*s1 + kn1
            nc.vector.tensor_scalar(out=tmp[:], in0=s1_nm[:], scalar1=-2.0,
                                    scalar2=1.0, op0=AOT.mult, op1=AOT.add)
            nc.vector.tensor_tensor(out=mse[:], in0=tmp[:], in1=kn1_nm[:],
                                    op=AOT.add)
            # d_ap = sqrt(mse + 2e-6*(qsum - ks1) + D*1e-12)
            nc.vector.tensor_tensor(out=dap[:], in0=qsum_nm[:], in1=ks1_nm[:],
                                    op=AOT.subtract)
            nc.vector.tensor_scalar(
                out=dap[:], in0=dap[:], scalar1=2.0 * EPS_TRIP,
                scalar2=float(D) * EPS_TRIP * EPS_TRIP,
                op0=AOT.mult, op1=AOT.add)
            nc.vector.tensor_tensor(out=dap[:], in0=dap[:], in1=mse[:],
                                    op=AOT.add)
            nc.scalar.sqrt(out=dap[:], in_=dap[:])
            # d_an = sqrt(1 - 2*s2 + kn2 + 2e-6*(qsum - ks2) + D*1e-12)
            nc.vector.tensor_scalar(out=tmp[:], in0=s2_nm[:], scalar1=-2.0,
                                    scalar2=1.0, op0=AOT.mult, op1=AOT.add)
            nc.vector.tensor_tensor(out=tmp[:], in0=tmp[:], in1=kn2_nm[:],
                                    op=AOT.add)
            nc.vector.tensor_tensor(out=dan[:], in0=qsum_nm[:], in1=ks2_nm[:],
                                    op=AOT.subtract)
            nc.vector.tensor_scalar(
                out=dan[:], in0=dan[:], scalar1=2.0 * EPS_TRIP,
                scalar2=float(D) * EPS_TRIP * EPS_TRIP,
                op0=AOT.mult, op1=AOT.add)
            nc.vector.tensor_tensor(out=dan[:], in0=dan[:], in1=tmp[:],
                                    op=AOT.add)
            nc.scalar.sqrt(out=dan[:], in_=dan[:])
            # triplet = relu(dap - dan + margin)
            nc.vector.tensor_tensor(out=trip[:], in0=dap[:], in1=dan[:],
                                    op=AOT.subtract)
            nc.scalar.activation(out=trip[:], in_=trip[:], func=AFT.Relu,
                                 bias=MARGIN, scale=1.0)

            # batch means: cols (b0:mse, b0:trip, b1:mse, b1:trip)
            lcols = persist.tile([128, 4], F32, tag="lcols", name="lcols")
            TB = NT // BL  # tiles per batch = 8
            for b in range(BL):
                nc.vector.reduce_sum(
                    out=lcols[:, 2 * b:2 * b + 1],
                    in_=mse[:, b * TB:(b + 1) * TB], axis=mybir.AxisListType.X)
                nc.vector.reduce_sum(
                    out=lcols[:, 2 * b + 1:2 * b + 2],
                    in_=trip[:, b * TB:(b + 1) * TB], axis=mybir.AxisListType.X)
            lps = psD.tile([1, 4], F32, tag="lps", name="lps")
            nc.tensor.matmul(out=lps[:], lhsT=ones_col[:], rhs=lcols[:],
                             start=True, stop=True)
            lrow = persist.tile([1, 4], F32, tag="lrow", name="lrow")
            nc.scalar.mul(out=lrow[:], in_=lps[:], mul=1.0 / float(HW))
            nc.sync.dma_start(out=loss_o[:].rearrange("a b -> () (a b)"),
                              in_=lrow[:])


_NC_CACHE = None


def _get_nc():
    global _NC_CACHE
    if _NC_CACHE is None:
        nc = build_nc()
        nc.finalize()  # Bacc: run wait-splitting + register allocation passes
        _NC_CACHE = nc
    return _NC_CACHE


def kernel(query: np.ndarray, keys_b: np.ndarray):
    from concourse.bass_utils import run_bass_kernel_spmd

    query = np.ascontiguousarray(query, dtype=np.float32)
    keys = np.ascontiguousarray(keys_b[0], dtype=np.float32)

    nc = _get_nc()
    in_maps = []
    for c in range(NCORES):
        qc = np.ascontiguousarray(
            query[c * BL:(c + 1) * BL].reshape(BL, D, HW))
        in_maps.append({"qin": qc, "keys": keys})

    res = run_bass_kernel_spmd(nc, in_maps, core_ids=list(range(NCORES))).results

    uq_full = np.concatenate(
        [np.asarray(r["uq"]).reshape(BL, 2 * D, H, W) for r in res], axis=0)
    smb_full = np.concatenate(
        [np.asarray(r["smb"]).reshape(BL, H, W, M) for r in res], axis=0)
    loss_full = np.concatenate([np.asarray(r["loss_o"]) for r in res], axis=0)

    gmax = np.max(
        np.stack([np.asarray(r["cmax_o"]).T.reshape(M) for r in res]), axis=0)
    S = np.sum(np.stack([np.asarray(r["s_out"]) for r in res]), axis=0)
    upd = (S * np.exp(-gmax)[:, None]).astype(np.float32)
    upd_mem = np.broadcast_to(upd[None], (B, M, D))

    return uq_full, upd_mem, smb_full, loss_full
